# revision 1
# baseline (speedup 1.0000x reference)
"""Gaussian-kernel attention (out = x + alpha * exp(-r_sigma*d2(x_i,x_j)) @ x)
for B=4, T=4096, C=64 on 8 trn2 NeuronCores.

Sharding: core = b*2 + h handles batch b, query rows [h*2048, (h+1)*2048).
Each core receives x[b] ROTATED so its own query rows come first
(xf = roll(x[b], -h*2048, axis=0)); the kernel's query block is then the
static slice xf[0:2048], and key order is a permutation (sum over keys is
permutation-invariant, so results are unchanged).

Per-core algorithm (flash-attention style, K never hits HBM):
  stage 1:  S[s, t] = -r_sigma * d2[s, t]  via ONE bf16 matmul with augmented
            contraction dim 66:
              A (66, T):    rows 0:64 = 2*r_sigma * x^T, row 64 = ones,
                            row 65 = -r_sigma * |x_s|^2
              R (66, ROWS): rows 0:64 = x^T (query cols), row 64 =
                            -r_sigma*|x_t|^2, row 65 = ones
            => A.T @ R = 2 r g - r sq_s - r sq_t = -r_sigma * d2
  stage 2:  K = exp(S) on ScalarE (PSUM -> SBUF, bf16, 1024-wide tiles)
            OT[c, t] += (alpha*x_chunk)^T @ K_chunk   (PSUM f32 accumulate)
  epilogue: out_ct = x^T[:, 0:ROWS] + OT  (stored (C, ROWS); host transposes)

For the actual operating point (r_sigma = 0) stage 1 is exactly zero in any
dtype (every product has a 0.0 or 1.0 operand), so K = 1.0 exactly; bf16 in
stage 2 only rounds x once (~3e-4 scale-relative output error).
The reference clamps d2 at 0 before exp; d2 >= 0 mathematically, so the clamp
only suppresses ~1e-6-scale rounding noise and is skipped here.
"""

import numpy as np

B, T, C = 4, 4096, 64
NCORES = 8
ROWS = T // 2        # query rows per core
TB = 1024            # t-block width (one exp tile; 2 PSUM banks)
SC = 128             # s-chunk (keys per inner step)
NSC = T // SC        # 32
NTB = ROWS // TB     # 2
MMN = 512            # max matmul free dim (one PSUM bank of f32)

_CACHE = {}


def _build_program():
    from contextlib import ExitStack

    import concourse.bass as bass  # noqa: F401
    import concourse.mybir as mybir
    import concourse.tile as tile
    from concourse import bacc
    from concourse.masks import make_identity

    f32 = mybir.dt.float32
    f32r = mybir.dt.float32r
    bf16 = mybir.dt.bfloat16
    Exp = mybir.ActivationFunctionType.Exp

    nc = bacc.Bacc(None, target_bir_lowering=False)
    xf = nc.dram_tensor("xf", (T, C), f32, kind="ExternalInput")
    rsig = nc.dram_tensor("rsig", (1, 1), f32, kind="ExternalInput")
    alp = nc.dram_tensor("alp", (1, 1), f32, kind="ExternalInput")
    out = nc.dram_tensor("out_ct", (C, ROWS), f32, kind="ExternalOutput")

    with ExitStack() as ctx:
        tc = ctx.enter_context(tile.TileContext(nc))
        cp = ctx.enter_context(tc.tile_pool(name="const", bufs=1))

        # ---- loads ----
        ident = cp.tile([128, 128], f32)
        make_identity(nc, ident)

        # x chunked: sb[p, si*C + c] = x[si*128 + p, c]
        xf_sb = cp.tile([128, NSC * C], f32)
        nc.sync.dma_start(
            xf_sb.rearrange("p (n c) -> p n c", c=C),
            xf.rearrange("(n p) c -> p n c", p=128),
        )
        rsig_sb = cp.tile([1, 1], f32)
        nc.sync.dma_start(rsig_sb[:], rsig[:])
        alp_sb = cp.tile([1, 1], f32)
        nc.sync.dma_start(alp_sb[:], alp[:])

        ones_f32 = cp.tile([1, T], f32)
        nc.vector.memset(ones_f32, 1.0)
        ones_col_f = cp.tile([64, 1], f32)
        nc.vector.memset(ones_col_f, 1.0)
        ones_col = cp.tile([64, 1], f32r)
        nc.vector.tensor_copy(ones_col, ones_col_f)

        # ---- broadcast scalars across partitions (via tiny matmuls) ----
        two_rsig = cp.tile([64, 1], f32)
        alpha_b = cp.tile([128, 1], f32)
        neg_rsig = cp.tile([1, 1], f32)
        with tc.tile_pool(name="bc", bufs=1, space="PSUM") as bc:
            rs_ps = bc.tile([64, 1], f32)
            nc.tensor.matmul(rs_ps, ones_f32[0:1, :64], rsig_sb[:],
                             start=True, stop=True)
            al_ps = bc.tile([128, 1], f32)
            nc.tensor.matmul(al_ps, ones_f32[0:1, :128], alp_sb[:],
                             start=True, stop=True)
            nc.vector.tensor_scalar_mul(two_rsig, rs_ps, 2.0)
            nc.vector.tensor_copy(alpha_b, al_ps)
            nc.vector.tensor_scalar_mul(neg_rsig, rsig_sb, -1.0)

        xa_sb = cp.tile([128, NSC * C], bf16)
        nc.vector.tensor_scalar_mul(xa_sb, xf_sb, alpha_b)

        # ---- transposes, aug matrices, squared norms ----
        # A rows: 0:64 = 2*r_sigma*x^T, 64 = ones, 65 = -r_sigma*sq
        # R rows: 0:64 = x^T[:, :ROWS], 64 = -r_sigma*sq[:ROWS], 65 = ones
        A_sb = cp.tile([66, T], bf16)
        R_sb = cp.tile([66, ROWS], bf16)
        xfT_raw = cp.tile([64, T], f32)    # exact x^T (epilogue + squares)
        xsq = cp.tile([64, T], f32r)
        # rows at partition 65 can't be written by DVE (start partition must
        # be 0/32/64/96): stage at partition 0, then SBUF->SBUF DMA.
        stage_sq = cp.tile([1, T], bf16)
        stage_ones = cp.tile([1, ROWS], bf16)
        nc.vector.tensor_copy(stage_ones, ones_f32[0:1, :ROWS])

        with (
            tc.tile_pool(name="tp", bufs=4, space="PSUM") as tpp,
            tc.tile_pool(name="sq", bufs=2, space="PSUM") as sqp,
        ):
            for si in range(NSC):
                tp = tpp.tile([64, 128], f32)
                nc.tensor.transpose(tp, xf_sb[:, si * C:(si + 1) * C], ident[:])
                sl = slice(si * 128, (si + 1) * 128)
                nc.vector.tensor_scalar_mul(A_sb[0:64, sl], tp, two_rsig)
                nc.vector.tensor_copy(xfT_raw[:, sl], tp)
                if si < NSC // 2:
                    nc.vector.tensor_copy(R_sb[0:64, sl], tp)

            # squared-norm row via ones-matmul (reduce over partitions)
            for j in range(T // MMN):
                sl = slice(j * MMN, (j + 1) * MMN)
                nc.vector.tensor_mul(xsq[:, sl], xfT_raw[:, sl], xfT_raw[:, sl])
                sp = sqp.tile([1, MMN], f32)
                nc.tensor.matmul(sp, ones_col[:], xsq[:, sl],
                                 start=True, stop=True)
                nc.vector.tensor_scalar_mul(stage_sq[0:1, sl], sp, neg_rsig)

        nc.vector.tensor_copy(A_sb[64:65, :], ones_f32)
        for j in range(T // MMN):
            sl = slice(j * MMN, (j + 1) * MMN)
            nc.sync.dma_start(A_sb[65:66, sl], stage_sq[0:1, sl])
        nc.sync.dma_start(R_sb[64:65, :], stage_sq[0:1, :ROWS])
        nc.sync.dma_start(R_sb[65:66, :], stage_ones[:])

        # ---- main loop ----
        with (
            tc.tile_pool(name="s_ps", bufs=2, space="PSUM") as spool,
            tc.tile_pool(name="o_ps", bufs=2, space="PSUM") as opool,
            tc.tile_pool(name="k_sb", bufs=3) as kpool,
            tc.tile_pool(name="r_sb", bufs=2) as rpool,
        ):
            for tb in range(NTB):
                tsl = slice(tb * TB, (tb + 1) * TB)
                ot = opool.tile([64, TB], f32)
                for si in range(NSC):
                    ssl = slice(si * SC, (si + 1) * SC)
                    csl = slice(si * C, (si + 1) * C)
                    s_ps = spool.tile([SC, TB], f32)
                    for h in range(TB // MMN):
                        nc.tensor.matmul(
                            s_ps[:, h * MMN:(h + 1) * MMN],
                            A_sb[:, ssl],
                            R_sb[:, tb * TB + h * MMN:tb * TB + (h + 1) * MMN],
                            start=True, stop=True,
                        )
                    k_sb = kpool.tile([SC, TB], bf16)
                    nc.scalar.activation(k_sb, s_ps, Exp)
                    for h in range(TB // MMN):
                        hs = slice(h * MMN, (h + 1) * MMN)
                        nc.tensor.matmul(
                            ot[:, hs], xa_sb[:, csl], k_sb[:, hs],
                            start=(si == 0), stop=(si == NSC - 1),
                        )
                res = rpool.tile([64, TB], f32)
                nc.vector.tensor_add(res, xfT_raw[:, tsl], ot)
                nc.sync.dma_start(out[:, tsl], res)

    return nc


def _get_program():
    if "nc" not in _CACHE:
        nc = _build_program()
        if not nc.is_finalized():
            nc.finalize()  # runs Bacc legalization (wait splitting, reg alloc)
        _CACHE["nc"] = nc
    return _CACHE["nc"]


def _make_in_maps(x, r_sigma, alpha):
    x = np.asarray(x, np.float32)
    rs = np.float32(np.asarray(r_sigma).reshape(())).reshape(1, 1)
    al = np.float32(np.asarray(alpha).reshape(())).reshape(1, 1)
    in_maps = []
    for core in range(NCORES):
        b, h = divmod(core, 2)
        xrot = np.roll(x[b], -h * ROWS, axis=0)
        in_maps.append({
            "xf": np.ascontiguousarray(xrot),
            "rsig": np.ascontiguousarray(rs),
            "alp": np.ascontiguousarray(al),
        })
    return in_maps


def kernel_with_results(x, r_sigma, alpha, trace=False):
    from concourse.bass_utils import run_bass_kernel_spmd

    nc = _get_program()
    res = run_bass_kernel_spmd(
        nc, _make_in_maps(x, r_sigma, alpha), core_ids=list(range(NCORES)),
        trace=trace,
    )
    out = np.empty((B, T, C), np.float32)
    for core in range(NCORES):
        b, h = divmod(core, 2)
        out[b, h * ROWS:(h + 1) * ROWS] = res.results[core]["out_ct"].T
    return out, res


def kernel(x, r_sigma, alpha):
    out, _ = kernel_with_results(x, r_sigma, alpha)
    return out



# revision 5
# speedup vs baseline: 1.0086x; 1.0086x over previous
"""Gaussian-kernel attention (out = x + alpha * exp(-r_sigma*d2(x_i,x_j)) @ x)
for B=4, T=4096, C=64 on 8 trn2 NeuronCores.

Sharding: core = b*2 + h handles batch b, query rows [h*2048, (h+1)*2048).
Each core receives x[b] ROTATED so its own query rows come first
(xf = roll(x[b], -h*2048, axis=0)); the kernel's query block is then the
static slice xf[0:2048], and key order is a permutation (sum over keys is
permutation-invariant, so results are unchanged).

Per-core algorithm (flash-attention style, K never hits HBM):
  S0[s, t] = <x_s, x_t>  via ONE bf16 gram matmul (contraction 64)
  K0       = exp(2r * S0 - r*|x_s|^2)  on ScalarE with per-partition bias
             (bias = -r*sq[s-chunk], scale = 2r, both runtime APs)
  OT0[c,t] += (alpha*x_chunk)^T @ K0_chunk  accumulated in PSUM; pairs of
             s-chunks run CONCURRENTLY in array column-groups via
             tile_position (0,0)/(0,64) (separate PSUM tiles per group).
  epilogue: out_ct = x^T[:, t] + E[t] * (OT0_even + OT0_odd)
            where E = exp(-r*|x_t|^2) broadcast to 64 partitions by a
            ones-matmul.  (exp(-r*sq_t) is constant over s, so it factors
            out of the key sum.)

For the actual operating point (r_sigma = 0): scale = 0 and bias = 0 make
K0 = exp(0) = 1.0 exactly (in any dtype), E = 1.0 exactly; bf16 only
rounds x once (~1e-3-scale output error).  The reference clamps d2 at 0
before exp; d2 >= 0 mathematically, so the clamp only suppresses
~1e-6-scale rounding noise and is skipped here (same as d2-form algebra).
"""

import numpy as np

B, T, C = 4, 4096, 64
NCORES = 8
ROWS = T // 2        # query rows per core
TB = 1024            # t-block width per pass (2 passes)
NTB = ROWS // TB     # 2
SC = 128             # s-chunk (keys per inner step)
NSC = T // SC        # 32
MMN = 512            # max matmul free dim (one PSUM bank of f32)

_CACHE = {}


def _build_program():
    from contextlib import ExitStack

    import concourse.bass as bass  # noqa: F401
    import concourse.mybir as mybir
    import concourse.tile as tile
    from concourse import bacc
    from concourse.masks import make_identity

    f32 = mybir.dt.float32
    f32r = mybir.dt.float32r
    bf16 = mybir.dt.bfloat16
    Exp = mybir.ActivationFunctionType.Exp

    nc = bacc.Bacc(None, target_bir_lowering=False)
    xf = nc.dram_tensor("xf", (T, C), f32, kind="ExternalInput")
    rsig = nc.dram_tensor("rsig", (1, 1), f32, kind="ExternalInput")
    alp = nc.dram_tensor("alp", (1, 1), f32, kind="ExternalInput")
    out = nc.dram_tensor("out_ct", (C, ROWS), f32, kind="ExternalOutput")

    with ExitStack() as ctx:
        tc = ctx.enter_context(tile.TileContext(nc))
        cp = ctx.enter_context(tc.tile_pool(name="const", bufs=1))

        # ---- input DMAs ----
        # x chunked: xf_sb[p, si*C + c] = x[si*128 + p, c]
        xf_sb = cp.tile([128, NSC * C], f32)
        nc.sync.dma_start(
            xf_sb.rearrange("p (n c) -> p n c", c=C),
            xf.rearrange("(n p) c -> p n c", p=128),
        )
        rsig_sb = cp.tile([1, 1], f32)
        nc.sync.dma_start(rsig_sb[:], rsig[:])
        alp_sb = cp.tile([1, 1], f32)
        nc.sync.dma_start(alp_sb[:], alp[:])

        ident = cp.tile([128, 128], f32)
        make_identity(nc, ident)
        ones_row = cp.tile([1, 128], f32)
        nc.vector.memset(ones_row, 1.0)
        ones_col_f = cp.tile([64, 1], f32)
        nc.vector.memset(ones_col_f, 1.0)
        ones_col = cp.tile([64, 1], f32r)
        nc.vector.tensor_copy(ones_col, ones_col_f)
        ones_col_b = cp.tile([1, 64], bf16)
        nc.vector.tensor_copy(ones_col_b, ones_row[0:1, 0:64])

        # warm the exp table set early (overlaps the big DMA)
        warm = cp.tile([1, 1], f32)
        nc.scalar.activation(warm, rsig_sb, Exp)

        # ---- broadcast runtime scalars across partitions ----
        two_r = cp.tile([128, 1], f32)     # 2*r_sigma  (ACT scale)
        negr = cp.tile([128, 1], f32)      # -r_sigma
        alpha_b = cp.tile([128, 1], f32)
        # SBUF tensors the main loop consumes
        xT_bf = cp.tile([64, T], bf16)       # x^T (keys+queries), bf16
        xfT_q = cp.tile([64, ROWS], f32)     # x^T (queries), exact f32
        xa = cp.tile([128, NSC * C], bf16)   # alpha * x, chunk layout
        xsq_full = cp.tile([128, NSC * C], f32)
        sq_col = cp.tile([128, NSC], f32)    # |x_s|^2 per chunk column
        nsq_col = cp.tile([128, NSC], f32)   # -r * |x_s|^2 (ACT bias)
        xsqT = cp.tile([64, ROWS], f32r)
        e_row = cp.tile([1, ROWS], bf16)     # exp(-r*|x_t|^2), queries

        with (
            tc.tile_pool(name="spool", bufs=2, space="PSUM") as spool,
            tc.tile_pool(name="opool", bufs=1, space="PSUM") as opool,
            tc.tile_pool(name="kpool", bufs=6) as kpool,
            tc.tile_pool(name="rpool", bufs=2) as rpool,
        ):
            # scalar broadcasts (tiny matmuls into a shared PSUM slot)
            bc_ps = spool.tile([128, 1], f32, name="bc_ps", tag="s")
            nc.tensor.matmul(bc_ps, ones_row[:], rsig_sb[:],
                             start=True, stop=True)
            nc.vector.tensor_scalar_mul(two_r, bc_ps, 2.0)
            nc.vector.tensor_scalar_mul(negr, bc_ps, -1.0)
            bc2_ps = spool.tile([128, 1], f32, name="bc2_ps", tag="s")
            nc.tensor.matmul(bc2_ps, ones_row[:], alp_sb[:],
                             start=True, stop=True)
            nc.vector.tensor_copy(alpha_b, bc2_ps)

            # ---- transposes: xf chunks -> x^T;  4 chunks per PSUM batch ----
            for bi in range(NSC // 4):
                tp = spool.tile([64, 512], f32, name="tp", tag="s")
                for k in range(4):
                    si = bi * 4 + k
                    nc.tensor.transpose(
                        tp[:, k * 128:(k + 1) * 128],
                        xf_sb[:, si * C:(si + 1) * C], ident[:])
                sl = slice(bi * 512, (bi + 1) * 512)
                nc.vector.tensor_copy(xT_bf[:, sl], tp)
                if bi < NSC // 8:   # query half also kept in f32
                    nc.vector.tensor_copy(xfT_q[:, sl], tp)

            # ---- alpha*x (bf16), squared norms ----
            nc.vector.tensor_scalar_mul(xa, xf_sb, alpha_b)
            nc.vector.tensor_mul(xsq_full, xf_sb, xf_sb)
            nc.vector.tensor_reduce(
                sq_col, xsq_full.rearrange("p (n c) -> p n c", c=C),
                axis=mybir.AxisListType.X, op=mybir.AluOpType.add)
            nc.vector.tensor_scalar_mul(nsq_col, sq_col, negr)

            # sq over queries as a row -> e_row = exp(-r*sq_t)
            nc.vector.tensor_mul(xsqT, xfT_q, xfT_q)
            for j in range(ROWS // MMN):
                sl = slice(j * MMN, (j + 1) * MMN)
                sqp = spool.tile([1, MMN], f32, name="sqp", tag="s")
                nc.tensor.matmul(sqp, ones_col[:], xsqT[:, sl],
                                 start=True, stop=True)
                nc.scalar.activation(e_row[0:1, sl], sqp, Exp,
                                     scale=negr[0:1, :])

            # ---- main loop ----
            for tb in range(NTB):
                tsl = slice(tb * TB, (tb + 1) * TB)
                ot_e = opool.tile([128, TB], f32, name="ot_e", tag="ot_e")
                ot_o = opool.tile([128, TB], f32, name="ot_o", tag="ot_o")
                kprev = None
                for si in range(NSC):
                    s_ps = spool.tile([SC, TB], f32, name="s_ps", tag="s")
                    for h in range(TB // MMN):
                        nc.tensor.matmul(
                            s_ps[:, h * MMN:(h + 1) * MMN],
                            xT_bf[:, si * SC:(si + 1) * SC],
                            xT_bf[:, tb * TB + h * MMN:tb * TB + (h + 1) * MMN],
                            start=True, stop=True)
                    k_sb = kpool.tile([SC, TB], bf16, name="k_sb")
                    nc.scalar.activation(k_sb, s_ps, Exp,
                                         bias=nsq_col[:, si:si + 1],
                                         scale=two_r)
                    if si % 2 == 0:
                        kprev = k_sb
                        continue
                    j = si // 2
                    first, last = (j == 0), (j == NSC // 2 - 1)
                    for h in range(TB // MMN):
                        hs = slice(h * MMN, (h + 1) * MMN)
                        nc.tensor.matmul(
                            ot_e[0:64, hs], xa[:, (si - 1) * C:si * C],
                            kprev[:, hs], start=first, stop=last,
                            tile_position=(0, 0))
                        nc.tensor.matmul(
                            ot_o[64:128, hs], xa[:, si * C:(si + 1) * C],
                            k_sb[:, hs], start=first, stop=last,
                            tile_position=(0, 64))

                # E = broadcast of e_row over 64 partitions (ones-matmul)
                e_ps = spool.tile([64, TB], f32, name="e_ps", tag="s")
                for h in range(TB // MMN):
                    hs = slice(h * MMN, (h + 1) * MMN)
                    nc.tensor.matmul(
                        e_ps[:, hs], ones_col_b[:],
                        e_row[0:1, tb * TB + h * MMN:tb * TB + (h + 1) * MMN],
                        start=True, stop=True)

                # partition-shifted copy (base 64 -> 0), then same-lane add
                oto_sb = rpool.tile([64, TB], f32, name="oto_sb")
                nc.vector.tensor_copy(oto_sb, ot_o[64:128, :])
                osum = rpool.tile([64, TB], f32, name="osum")
                nc.vector.tensor_add(osum, ot_e[0:64, :], oto_sb)
                oscl = rpool.tile([64, TB], f32, name="oscl")
                nc.vector.tensor_mul(oscl, osum, e_ps)
                res = rpool.tile([64, TB], f32, name="res")
                nc.vector.tensor_add(res, xfT_q[:, tsl], oscl)
                nc.sync.dma_start(out[:, tsl], res)

    return nc


def _get_program():
    if "nc" not in _CACHE:
        nc = _build_program()
        if not nc.is_finalized():
            nc.finalize()  # runs Bacc legalization (wait splitting, reg alloc)
        _CACHE["nc"] = nc
    return _CACHE["nc"]


def _make_in_maps(x, r_sigma, alpha):
    x = np.asarray(x, np.float32)
    rs = np.float32(np.asarray(r_sigma).reshape(())).reshape(1, 1)
    al = np.float32(np.asarray(alpha).reshape(())).reshape(1, 1)
    in_maps = []
    for core in range(NCORES):
        b, h = divmod(core, 2)
        xrot = np.roll(x[b], -h * ROWS, axis=0)
        in_maps.append({
            "xf": np.ascontiguousarray(xrot),
            "rsig": np.ascontiguousarray(rs),
            "alp": np.ascontiguousarray(al),
        })
    return in_maps


def kernel_with_results(x, r_sigma, alpha, trace=False):
    from concourse.bass_utils import run_bass_kernel_spmd

    nc = _get_program()
    res = run_bass_kernel_spmd(
        nc, _make_in_maps(x, r_sigma, alpha), core_ids=list(range(NCORES)),
        trace=trace,
    )
    out = np.empty((B, T, C), np.float32)
    for core in range(NCORES):
        b, h = divmod(core, 2)
        out[b, h * ROWS:(h + 1) * ROWS] = res.results[core]["out_ct"].T
    return out, res


def kernel(x, r_sigma, alpha):
    out, _ = kernel_with_results(x, r_sigma, alpha)
    return out
